# revision 26
# baseline (speedup 1.0000x reference)
"""Trainium2 Bass kernel for Llama-style GQA attention block (B=1, S=2048,
D=4096, 32 q heads / 8 kv heads, head_dim 128, neox RoPE, causal).

Sharding: tensor-parallel over kv heads across 8 NeuronCores. Core c gets
kv head c and q heads [4c, 4c+4). Each core computes a full [S, D] partial
of the output (o_proj row-parallel); host sums the 8 partials.

v2 (bf16 + fused schedule): all matmul operands bf16 (fp32 PSUM accumulate),
weights streamed in chunks so the first projection matmul starts ~3us in,
o_proj blocks of q-slice qs-1 interleaved into the attention t-loop of qs so
the PE never waits on softmax exp latency, causal mask applied as a 0/1
multiply after exp (cheap bf16 DVE op), denominators accumulated on DVE and
partition-replicated with a ones-matmul, reciprocal via the fast DVE approx.

Per-core phases (one TileContext, dependency-scheduled):
  A) per S-slice of 512: qT/kT/vT projections (weights stationary, hiddenT
     streamed), then neox RoPE on the slice via rotate-half PE matmul +
     cos/sin DVE ops, then V natural-layout tiles via PE transposes.
  B) per q-slice: flash-style attention in transposed-score layout
     (scoresT = kT_tile.T @ qT_slice), exp on ACT (scale folded in),
     staircase 0/1 mask multiply on diagonal tiles, O^T accumulated in PSUM
     with V stationary, denominator = ones-matmul partition sum.
  C) o_proj matmuls for q-slice qs-1 emitted inside attention of qs.
"""

import threading
from dataclasses import dataclass

import numpy as np


@dataclass(frozen=True)
class Cfg:
    S: int = 2048      # sequence length
    D: int = 4096      # hidden size
    HQ: int = 4        # q heads per core
    DH: int = 128      # head dim
    QSL: int = 512     # q-slice width (= matmul N)
    theta: float = 10000.0
    cores: int = 8


FULL = Cfg()


def build_nc(cfg: Cfg):
    import concourse.bass as bass  # noqa: F401
    import concourse.mybir as mybir
    import concourse.tile as tile
    from concourse import bacc

    F32 = mybir.dt.float32
    F32R = mybir.dt.float32r
    BF16 = mybir.dt.bfloat16

    S, D, HQ, DH, QSL = cfg.S, cfg.D, cfg.HQ, cfg.DH, cfg.QSL
    DT = D // 128          # d (contraction) tiles
    NKT = S // 128         # k position tiles
    NQS = S // QSL         # q slices
    NDT = D // 128         # output D row-tiles (phase C)
    scale = float(DH) ** -0.5
    Exp = mybir.ActivationFunctionType.Exp

    nc = bacc.Bacc("TRN2", target_bir_lowering=False, debug=False,
                   num_devices=cfg.cores)

    hT = nc.dram_tensor("hT", [D, S], BF16, kind="ExternalInput").ap()
    wq = nc.dram_tensor("wq", [D, HQ * DH], BF16, kind="ExternalInput").ap()
    wk = nc.dram_tensor("wk", [D, DH], BF16, kind="ExternalInput").ap()
    wv = nc.dram_tensor("wv", [D, DH], BF16, kind="ExternalInput").ap()
    wo = nc.dram_tensor("wo", [HQ * DH, D], BF16, kind="ExternalInput").ap()
    cosT = nc.dram_tensor("cosT", [DH, S], BF16, kind="ExternalInput").ap()
    sinT = nc.dram_tensor("sinT", [DH, S], BF16, kind="ExternalInput").ap()
    binm = nc.dram_tensor("binm", [128, 2 * QSL], BF16,
                          kind="ExternalInput").ap()
    cstb = nc.dram_tensor("cstb", [128, 256], BF16, kind="ExternalInput").ap()
    onesf = nc.dram_tensor("onesf", [128, 128], F32R, kind="ExternalInput").ap()
    outT = nc.dram_tensor("outT", [D, S], F32, kind="ExternalOutput").ap()

    with tile.TileContext(nc) as tc, \
            tc.tile_pool(name="main", bufs=1) as pm, \
            tc.tile_pool(name="expp", bufs=16) as ep:
        # long-lived SBUF tensors
        qT = [pm.tile([128, S], BF16, tag=f"qT{g}", name=f"qT{g}")
              for g in range(HQ)]
        kT = pm.tile([128, S], BF16, tag="kT")
        vT = pm.tile([128, S], BF16, tag="vT")
        v_all = pm.tile([128, NKT, DH], BF16, tag="vall")
        o_attn = [pm.tile([128, S], BF16, tag=f"oT{g}", name=f"oT{g}")
                  for g in range(HQ)]
        cos_sb = pm.tile([128, S], BF16, tag="cos")
        sin_sb = pm.tile([128, S], BF16, tag="sin")
        binm_sb = pm.tile([128, 2 * QSL], BF16, tag="binm")
        cst_sb = pm.tile([128, 256], BF16, tag="cstb")
        ones_sb = pm.tile([128, 128], F32R, tag="ones")
        wq_sb = pm.tile([128, DT, HQ * DH], BF16, tag="wq")
        wk_sb = pm.tile([128, DT, DH], BF16, tag="wk")
        wv_sb = pm.tile([128, DT, DH], BF16, tag="wv")
        wo_sb = pm.tile([128, HQ, D], BF16, tag="wo")

        rot_sb = cst_sb[:, 0:128]
        ident = cst_sb[:, 128:256]

        # ---- prologue DMAs ----
        # scalar (qActDynamicHW) ring: weights in d-chunks so the first
        # phase-A matmul only waits for chunk 0; wo last (needed from C(0)).
        wq_r = wq.rearrange("(t p) m -> p t m", p=128)
        wk_r = wk.rearrange("(t p) m -> p t m", p=128)
        wv_r = wv.rearrange("(t p) m -> p t m", p=128)
        chunks = [(0, 1), (1, 3), (3, 7), (7, 15), (15, 23), (23, 32)]
        for c0, c1 in chunks:
            cs = slice(c0, c1)
            nc.scalar.dma_start(out=wq_sb[:, cs, :], in_=wq_r[:, cs, :])
            nc.scalar.dma_start(out=wk_sb[:, cs, :], in_=wk_r[:, cs, :])
            nc.scalar.dma_start(out=wv_sb[:, cs, :], in_=wv_r[:, cs, :])
        nc.scalar.dma_start(out=cos_sb, in_=cosT)
        nc.scalar.dma_start(out=sin_sb, in_=sinT)
        nc.scalar.dma_start(out=binm_sb, in_=binm)
        nc.scalar.dma_start(out=cst_sb, in_=cstb)
        nc.scalar.dma_start(out=ones_sb, in_=onesf)
        nc.scalar.dma_start(out=wo_sb,
                            in_=wo.rearrange("(t p) n -> p t n", p=128))

        # Zero-prime the exp-pool slots once: diagonal-tile exps write only
        # [delta:], and the 0/1 mask multiply reads the whole tile — stale
        # bytes must be finite (0 * garbage = 0 needs garbage != NaN/Inf).
        prime = [ep.tile([128, QSL], BF16, tag="exp", name=f"prime{i}")
                 for i in range(16)]
        for pt_ in prime:
            nc.gpsimd.memset(pt_, 0.0)

        def emit_sexp(pool, tag, qs, t, g):
            """Scores matmul + exp (+ causal mask) for one (qs, t, g) tile.
            Returns the bf16 exp tile."""
            delta = t * 128 - qs * QSL
            dlo = max(0, delta)
            qb = qs * QSL
            ktile = kT[:, t * 128:(t + 1) * 128]
            ps = pool.tile([128, QSL], F32, tag=tag, name="ps")
            nc.tensor.matmul(ps[:, dlo:], ktile, qT[g][:, qb + dlo:qb + QSL],
                             start=True, stop=True)
            ex = ep.tile([128, QSL], BF16, tag="exp", name="ex")
            nc.scalar.activation(ex[:, dlo:], ps[:, dlo:], Exp,
                                 bias=0.0, scale=scale)
            if delta >= 0:
                # zeroes the staircase AND the stale [0:dlo)
                nc.vector.tensor_mul(
                    ex, ex, binm_sb[:, QSL - delta:2 * QSL - delta])
            return ex

        # qs=0 score/exp tiles hoisted into phase A; keyed by (g, t)
        ex0 = {}
        hoist = [(g, t) for t in range(QSL // 128) for g in range(HQ)]

        # ---------------- phase A: projections + RoPE + V ----------------
        with (
            tc.tile_pool(name="hstream", bufs=8) as hp,
            tc.tile_pool(name="psA", bufs=6, space="PSUM") as psA,
            tc.tile_pool(name="psR", bufs=2, space="PSUM") as psR,
            tc.tile_pool(name="ropet", bufs=4) as rtp,
        ):
            hT_r = hT.rearrange("(t p) s -> p t s", p=128)
            groups = [(wq_sb[:, :, g * DH:(g + 1) * DH], qT[g])
                      for g in range(HQ)]
            groups.append((wk_sb, kT))
            groups.append((wv_sb, vT))
            NG = len(groups)
            for sl in range(NQS):
                ssl = slice(sl * QSL, (sl + 1) * QSL)
                pss = [psA.tile([128, QSL], F32, tag="psA", name="psA")
                       for _ in range(NG)]
                for d0 in range(0, DT, 2):
                    ht2 = hp.tile([128, 2, QSL], BF16, tag="ht")
                    nc.sync.dma_start(out=ht2, in_=hT_r[:, d0:d0 + 2, ssl])
                    for j in range(2):
                        d = d0 + j
                        for gi, (wtile, _) in enumerate(groups):
                            nc.tensor.matmul(pss[gi], wtile[:, d, :],
                                             ht2[:, j, :],
                                             start=(d == 0),
                                             stop=(d == DT - 1))
                    # hoist qs=0 attention score/exp tiles into the last two
                    # slices (RoPE(0) is done by then); fills ACT's idle time
                    if sl >= 2 and d0 % 4 == 2 and hoist:
                        g, t = hoist.pop(0)
                        ex0[(g, t)] = emit_sexp(psR, "pr", 0, t, g)
                for gi, (_, dst) in enumerate(groups):
                    if gi % 2 == 0:
                        nc.scalar.copy(dst[:, ssl], pss[gi])
                    else:
                        nc.vector.tensor_copy(dst[:, ssl], pss[gi])
                # RoPE on this slice of qT + kT:
                #   blk = blk*cos + rot(blk)*sin, rot via PE matmul.
                for blk in qT + [kT]:
                    pr = psR.tile([128, QSL], F32, tag="pr", name="pr")
                    nc.tensor.matmul(pr, rot_sb, blk[:, ssl],
                                     start=True, stop=True)
                    rs = rtp.tile([128, QSL], BF16, tag="rs")
                    nc.vector.tensor_mul(rs, pr, sin_sb[:, ssl])
                    cc = rtp.tile([128, QSL], BF16, tag="rc")
                    nc.vector.tensor_mul(cc, blk[:, ssl], cos_sb[:, ssl])
                    nc.vector.tensor_add(blk[:, ssl], cc, rs)
                # V natural-layout tiles for this slice
                for tt in range(QSL // 128):
                    t = sl * (QSL // 128) + tt
                    pt = psR.tile([128, 128], BF16, tag="pr", name="pt")
                    nc.tensor.transpose(pt, vT[:, t * 128:(t + 1) * 128],
                                        ident)
                    nc.scalar.copy(v_all[:, t, :], pt)

        # ------------- phase B/C: attention + interleaved o_proj -------------
        with (
            tc.tile_pool(name="psS", bufs=2, space="PSUM") as psS,
            tc.tile_pool(name="psO", bufs=4, space="PSUM") as psO,
            tc.tile_pool(name="psC", bufs=2, space="PSUM") as psC,
            tc.tile_pool(name="expp", bufs=8) as ep,
            tc.tile_pool(name="accp", bufs=8) as ap_,
            tc.tile_pool(name="dnp", bufs=2) as dp,
            tc.tile_pool(name="accr", bufs=4) as arp,
            tc.tile_pool(name="ocp", bufs=4) as ocp,
        ):
            # Prime the exp-pool slots with zeros once: diagonal-tile exps
            # write only [delta:], and the 0/1 mask multiply reads the whole
            # tile — stale bytes must be finite (0 * garbage = 0 needs
            # garbage != NaN/Inf).
            prime = [ep.tile([128, QSL], BF16, tag="exp", name=f"prime{i}")
                     for i in range(8)]
            for pt_ in prime:
                nc.gpsimd.memset(pt_, 0.0)

            cq = []  # pending o_proj blocks: (qs, Dt)

            def emit_c_block():
                qs, Dt = cq.pop(0)
                qsl = slice(qs * QSL, (qs + 1) * QSL)
                pf = psC.tile([128, QSL], F32, tag="psC", name="psC")
                for g in range(HQ):
                    nc.tensor.matmul(
                        pf, wo_sb[:, g, Dt * 128:(Dt + 1) * 128],
                        o_attn[g][:, qsl],
                        start=(g == 0), stop=(g == HQ - 1))
                oc = ocp.tile([128, QSL], F32, tag="oc")
                nc.scalar.copy(oc, pf)
                nc.sync.dma_start(
                    out=outT[Dt * 128:(Dt + 1) * 128, qsl], in_=oc)

            for qs in range(NQS):
                qsl = slice(qs * QSL, (qs + 1) * QSL)
                nkt = (qs + 1) * (QSL // 128)
                # o_proj blocks of qs-1: spread most across this q-slice's
                # t-loop, hold back a few to cover the softmax-denominator
                # chain at the end of the slice.
                reserve = min(4, len(cq))
                bpt = max(0, (len(cq) - reserve)) // nkt if cq else 0
                po = [psO.tile([128, QSL], F32, tag="psO", name="psO")
                      for _ in range(HQ)]
                acc = [ap_.tile([128, QSL], F32, tag="acc", name="acc")
                       for _ in range(HQ)]
                accr = [arp.tile([128, QSL], F32R, tag="accr", name="accr")
                        for _ in range(HQ)]
                for t in range(nkt):
                    nc1 = (bpt + 1) // 2
                    if qs == 0:
                        exs = [ex0.pop((g, t)) for g in range(HQ)]
                    else:
                        exs = []
                        for g in range(HQ):
                            exs.append(emit_sexp(psS, "psS", qs, t, g))
                            if g == 1:  # PE filler while exps run
                                for _ in range(nc1):
                                    if cq:
                                        emit_c_block()
                    for g in range(HQ):
                        nc.tensor.matmul(po[g], v_all[:, t, :], exs[g],
                                         start=(t == 0),
                                         stop=(t == nkt - 1))
                        if t == 0:
                            nc.vector.tensor_copy(acc[g], exs[g])
                        elif t == nkt - 1:
                            # final add rounds to f32r for the ones-matmul
                            nc.vector.tensor_add(accr[g], acc[g], exs[g])
                        elif g == 0:
                            nc.gpsimd.tensor_add(acc[g], acc[g], exs[g])
                        else:
                            nc.vector.tensor_add(acc[g], acc[g], exs[g])
                    for _ in range(bpt - nc1):
                        if cq:
                            emit_c_block()
                # softmax denominators + normalize into o_attn (bf16);
                # held-back o_proj blocks keep the PE fed meanwhile
                for g in range(HQ):
                    if cq:
                        emit_c_block()
                    pd = psS.tile([128, QSL], F32, tag="psS", name="pd")
                    nc.tensor.matmul(pd, ones_sb, accr[g],
                                     start=True, stop=True)
                    dn = dp.tile([128, QSL], F32, tag="dn")
                    nc.vector.reciprocal_approx_fast(dn, pd)
                    nc.vector.tensor_mul(o_attn[g][:, qsl], po[g], dn)
                cq.extend((qs, Dt) for Dt in range(NDT))
            while cq:
                emit_c_block()

    nc.compile()
    return nc


def make_tables(cfg: Cfg, position_ids: np.ndarray):
    """cosT/sinT [128, S]: row d holds cos/sin(pos * invfreq[d % 64])."""
    half = cfg.DH // 2
    inv = 1.0 / (cfg.theta ** (np.arange(half, dtype=np.float64) * 2.0 / cfg.DH))
    pos = np.asarray(position_ids).reshape(-1).astype(np.float64)  # [S]
    ang = inv[:, None] * pos[None, :]                              # [64, S]
    cosT = np.concatenate([np.cos(ang), np.cos(ang)], 0)
    sinT = np.concatenate([np.sin(ang), np.sin(ang)], 0)
    return cosT, sinT


def make_cst(cfg: Cfg):
    """[128, 256] cols 0-127: rotate-half stationary matrix (out = M^T @ x,
    out[:64] = -x[64:], out[64:] = x[:64]); cols 128-255: identity."""
    half = cfg.DH // 2
    m = np.zeros((128, 256), np.float64)
    for i in range(half):
        m[i + half, i] = -1.0
        m[i, i + half] = 1.0
    m[:, 128:256] = np.eye(128)
    return m


def make_binm(cfg: Cfg):
    """0/1 staircase [128, 2*QSL]: col c, row p -> 1 if (c - QSL) >= p else 0.
    Diagonal k-tile with offset delta uses cols [QSL-delta, 2*QSL-delta)."""
    c = np.arange(2 * cfg.QSL)[None, :] - cfg.QSL
    p = np.arange(128)[:, None]
    return (c >= p).astype(np.float64)


_cache = threading.Lock()
_nc_full = None


def _get_nc():
    global _nc_full
    with _cache:
        if _nc_full is None:
            _nc_full = build_nc(FULL)
    return _nc_full


def core_inputs(cfg: Cfg, c: int, position_ids, hidden_states, Wq, Wk, Wv, Wo):
    """Build the per-core input map (numpy, bf16 operands) for core c."""
    from ml_dtypes import bfloat16

    def bf(x):
        return np.ascontiguousarray(np.asarray(x).astype(bfloat16))

    S, D, HQ, DH = cfg.S, cfg.D, cfg.HQ, cfg.DH
    hT = np.asarray(hidden_states, dtype=np.float32).reshape(S, D).T
    cosT, sinT = make_tables(cfg, position_ids)
    qc = slice(c * HQ * DH, (c + 1) * HQ * DH)
    kc = slice(c * DH, (c + 1) * DH)
    return {
        "hT": bf(hT),
        "wq": bf(np.asarray(Wq, np.float32)[:, qc]),
        "wk": bf(np.asarray(Wk, np.float32)[:, kc]),
        "wv": bf(np.asarray(Wv, np.float32)[:, kc]),
        "wo": bf(np.asarray(Wo, np.float32)[qc, :]),
        "cosT": bf(cosT),
        "sinT": bf(sinT),
        "binm": bf(make_binm(cfg)),
        "cstb": bf(make_cst(cfg)),
        "onesf": np.ones((128, 128), np.float32),
    }


def kernel(position_ids, hidden_states, Wq, Wk, Wv, Wo, _trace=False):
    from concourse.bass_utils import run_bass_kernel_spmd

    cfg = FULL
    nc = _get_nc()
    args = (position_ids, hidden_states, Wq, Wk, Wv, Wo)
    in_maps = [core_inputs(cfg, c, *args) for c in range(cfg.cores)]
    res = run_bass_kernel_spmd(nc, in_maps, core_ids=list(range(cfg.cores)),
                               trace=_trace)
    out = np.zeros((cfg.S, cfg.D), np.float64)
    for c in range(cfg.cores):
        out += res.results[c]["outT"].T.astype(np.float64)
    ret = out.astype(np.float32).reshape(1, cfg.S, cfg.D)
    if _trace:
        return ret, res
    return ret


# revision 30
# speedup vs baseline: 1.1896x; 1.1896x over previous
"""Trainium2 Bass kernel for Llama-style GQA attention block (B=1, S=2048,
D=4096, 32 q heads / 8 kv heads, head_dim 128, neox RoPE, causal).

Sharding: tensor-parallel over kv heads across 8 NeuronCores. Core c gets
kv head c and q heads [4c, 4c+4). Each core computes a full [S, D] partial
of the output (o_proj row-parallel); host sums the 8 partials.

v2 (bf16 + fused schedule): all matmul operands bf16 (fp32 PSUM accumulate),
weights streamed in chunks so the first projection matmul starts ~3us in,
o_proj blocks of q-slice qs-1 interleaved into the attention t-loop of qs so
the PE never waits on softmax exp latency, causal mask applied as a 0/1
multiply after exp (cheap bf16 DVE op), denominators accumulated on DVE and
partition-replicated with a ones-matmul, reciprocal via the fast DVE approx.

Per-core phases (one TileContext, dependency-scheduled):
  A) per S-slice of 512: qT/kT/vT projections (weights stationary, hiddenT
     streamed), then neox RoPE on the slice via rotate-half PE matmul +
     cos/sin DVE ops, then V natural-layout tiles via PE transposes.
  B) per q-slice: flash-style attention in transposed-score layout
     (scoresT = kT_tile.T @ qT_slice), exp on ACT (scale folded in),
     staircase 0/1 mask multiply on diagonal tiles, O^T accumulated in PSUM
     with V stationary, denominator = ones-matmul partition sum.
  C) o_proj matmuls for q-slice qs-1 emitted inside attention of qs.
"""

import threading
from dataclasses import dataclass

import numpy as np


@dataclass(frozen=True)
class Cfg:
    S: int = 2048      # sequence length
    D: int = 4096      # hidden size
    HQ: int = 4        # q heads per core
    DH: int = 128      # head dim
    QSL: int = 512     # q-slice width (= matmul N)
    theta: float = 10000.0
    cores: int = 8


FULL = Cfg()


def build_nc(cfg: Cfg):
    import concourse.bass as bass  # noqa: F401
    import concourse.mybir as mybir
    import concourse.tile as tile
    from concourse import bacc

    F32 = mybir.dt.float32
    F32R = mybir.dt.float32r
    BF16 = mybir.dt.bfloat16

    S, D, HQ, DH, QSL = cfg.S, cfg.D, cfg.HQ, cfg.DH, cfg.QSL
    DT = D // 128          # d (contraction) tiles
    NKT = S // 128         # k position tiles
    NQS = S // QSL         # q slices
    NDT = D // 128         # output D row-tiles (phase C)
    scale = float(DH) ** -0.5
    Exp = mybir.ActivationFunctionType.Exp

    nc = bacc.Bacc("TRN2", target_bir_lowering=False, debug=False,
                   num_devices=cfg.cores)

    hT = nc.dram_tensor("hT", [D, S], BF16, kind="ExternalInput").ap()
    wq = nc.dram_tensor("wq", [D, HQ * DH], BF16, kind="ExternalInput").ap()
    wk = nc.dram_tensor("wk", [D, DH], BF16, kind="ExternalInput").ap()
    wv = nc.dram_tensor("wv", [D, DH], BF16, kind="ExternalInput").ap()
    wo = nc.dram_tensor("wo", [HQ * DH, D], BF16, kind="ExternalInput").ap()
    cosT = nc.dram_tensor("cosT", [DH, S], BF16, kind="ExternalInput").ap()
    sinT = nc.dram_tensor("sinT", [DH, S], BF16, kind="ExternalInput").ap()
    binm = nc.dram_tensor("binm", [128, 2 * QSL], BF16,
                          kind="ExternalInput").ap()
    cstb = nc.dram_tensor("cstb", [128, 256], BF16, kind="ExternalInput").ap()
    onesf = nc.dram_tensor("onesf", [128, 128], F32R, kind="ExternalInput").ap()
    outT = nc.dram_tensor("outT", [D, S], F32, kind="ExternalOutput").ap()

    with tile.TileContext(nc) as tc, \
            tc.tile_pool(name="main", bufs=1) as pm, \
            tc.tile_pool(name="expp", bufs=16) as ep:
        # long-lived SBUF tensors
        qT = [pm.tile([128, S], BF16, tag=f"qT{g}", name=f"qT{g}")
              for g in range(HQ)]
        kT = pm.tile([128, S], BF16, tag="kT")
        vT = pm.tile([128, S], BF16, tag="vT")
        v_all = pm.tile([128, NKT, DH], BF16, tag="vall")
        o_attn = [pm.tile([128, S], BF16, tag=f"oT{g}", name=f"oT{g}")
                  for g in range(HQ)]
        cos_sb = pm.tile([128, S], BF16, tag="cos")
        sin_sb = pm.tile([128, S], BF16, tag="sin")
        binm_sb = pm.tile([128, 2 * QSL], BF16, tag="binm")
        cst_sb = pm.tile([128, 256], BF16, tag="cstb")
        ones_sb = pm.tile([128, 128], F32R, tag="ones")
        wq_sb = pm.tile([128, DT, HQ * DH], BF16, tag="wq")
        wk_sb = pm.tile([128, DT, DH], BF16, tag="wk")
        wv_sb = pm.tile([128, DT, DH], BF16, tag="wv")
        wo_sb = pm.tile([128, HQ, D], BF16, tag="wo")

        rot_sb = cst_sb[:, 0:128]
        ident = cst_sb[:, 128:256]

        # ---- prologue DMAs ----
        # scalar (qActDynamicHW) ring: weights in d-chunks so the first
        # phase-A matmul only waits for chunk 0; wo last (needed from C(0)).
        wq_r = wq.rearrange("(t p) m -> p t m", p=128)
        wk_r = wk.rearrange("(t p) m -> p t m", p=128)
        wv_r = wv.rearrange("(t p) m -> p t m", p=128)
        chunks = [(0, 1), (1, 3), (3, 7), (7, 15), (15, 23), (23, 32)]
        for c0, c1 in chunks:
            cs = slice(c0, c1)
            nc.scalar.dma_start(out=wq_sb[:, cs, :], in_=wq_r[:, cs, :])
            nc.scalar.dma_start(out=wk_sb[:, cs, :], in_=wk_r[:, cs, :])
            nc.scalar.dma_start(out=wv_sb[:, cs, :], in_=wv_r[:, cs, :])
        nc.scalar.dma_start(out=cos_sb, in_=cosT)
        nc.scalar.dma_start(out=sin_sb, in_=sinT)
        nc.scalar.dma_start(out=binm_sb, in_=binm)
        nc.scalar.dma_start(out=cst_sb, in_=cstb)
        nc.scalar.dma_start(out=ones_sb, in_=onesf)
        nc.scalar.dma_start(out=wo_sb,
                            in_=wo.rearrange("(t p) n -> p t n", p=128))

        # Zero-prime the exp-pool slots once: diagonal-tile exps write only
        # [delta:], and the 0/1 mask multiply reads the whole tile — stale
        # bytes must be finite (0 * garbage = 0 needs garbage != NaN/Inf).
        prime = [ep.tile([128, QSL], BF16, tag="exp", name=f"prime{i}")
                 for i in range(16)]
        for pt_ in prime:
            nc.gpsimd.memset(pt_, 0.0)

        def emit_sexp(pool, tag, qs, t, g):
            """Scores matmul + exp (+ causal mask) for one (qs, t, g) tile.
            Returns the bf16 exp tile."""
            delta = t * 128 - qs * QSL
            dlo = max(0, delta)
            qb = qs * QSL
            ktile = kT[:, t * 128:(t + 1) * 128]
            ps = pool.tile([128, QSL], F32, tag=tag, name="ps")
            nc.tensor.matmul(ps[:, dlo:], ktile, qT[g][:, qb + dlo:qb + QSL],
                             start=True, stop=True)
            ex = ep.tile([128, QSL], BF16, tag="exp", name="ex")
            nc.scalar.activation(ex[:, dlo:], ps[:, dlo:], Exp,
                                 bias=0.0, scale=scale)
            if delta >= 0:
                # zeroes the staircase AND the stale [0:dlo)
                nc.vector.tensor_mul(
                    ex, ex, binm_sb[:, QSL - delta:2 * QSL - delta])
            return ex

        # ---------------- phase A: projections + RoPE + V ----------------
        with (
            tc.tile_pool(name="hstream", bufs=8) as hp,
            tc.tile_pool(name="psA", bufs=6, space="PSUM") as psA,
            tc.tile_pool(name="psR", bufs=2, space="PSUM") as psR,
            tc.tile_pool(name="ropet", bufs=4) as rtp,
        ):
            hT_r = hT.rearrange("(t p) s -> p t s", p=128)
            groups = [(wq_sb[:, :, g * DH:(g + 1) * DH], qT[g])
                      for g in range(HQ)]
            groups.append((wk_sb, kT))
            groups.append((wv_sb, vT))
            NG = len(groups)
            for sl in range(NQS):
                ssl = slice(sl * QSL, (sl + 1) * QSL)
                pss = [psA.tile([128, QSL], F32, tag="psA", name="psA")
                       for _ in range(NG)]
                for d0 in range(0, DT, 2):
                    ht2 = hp.tile([128, 2, QSL], BF16, tag="ht")
                    nc.sync.dma_start(out=ht2, in_=hT_r[:, d0:d0 + 2, ssl])
                    for j in range(2):
                        d = d0 + j
                        for gi, (wtile, _) in enumerate(groups):
                            nc.tensor.matmul(pss[gi], wtile[:, d, :],
                                             ht2[:, j, :],
                                             start=(d == 0),
                                             stop=(d == DT - 1))

                for gi, (_, dst) in enumerate(groups):
                    if gi % 2 == 0:
                        nc.scalar.copy(dst[:, ssl], pss[gi])
                    else:
                        nc.vector.tensor_copy(dst[:, ssl], pss[gi])
                # RoPE on this slice of qT + kT:
                #   blk = blk*cos + rot(blk)*sin, rot via PE matmul.
                for blk in qT + [kT]:
                    pr = psR.tile([128, QSL], F32, tag="pr", name="pr")
                    nc.tensor.matmul(pr, rot_sb, blk[:, ssl],
                                     start=True, stop=True)
                    rs = rtp.tile([128, QSL], BF16, tag="rs")
                    nc.vector.tensor_mul(rs, pr, sin_sb[:, ssl])
                    cc = rtp.tile([128, QSL], BF16, tag="rc")
                    nc.vector.tensor_mul(cc, blk[:, ssl], cos_sb[:, ssl])
                    nc.vector.tensor_add(blk[:, ssl], cc, rs)
                # V natural-layout tiles for this slice
                for tt in range(QSL // 128):
                    t = sl * (QSL // 128) + tt
                    pt = psR.tile([128, 128], BF16, tag="pr", name="pt")
                    nc.tensor.transpose(pt, vT[:, t * 128:(t + 1) * 128],
                                        ident)
                    nc.scalar.copy(v_all[:, t, :], pt)

        # ------------- phase B/C: attention + interleaved o_proj -------------
        with (
            tc.tile_pool(name="psS", bufs=2, space="PSUM") as psS,
            tc.tile_pool(name="psO", bufs=4, space="PSUM") as psO,
            tc.tile_pool(name="psC", bufs=2, space="PSUM") as psC,
            tc.tile_pool(name="expp", bufs=8) as ep,
            tc.tile_pool(name="accp", bufs=8) as ap_,
            tc.tile_pool(name="dnp", bufs=2) as dp,
            tc.tile_pool(name="accr", bufs=4) as arp,
            tc.tile_pool(name="ocp", bufs=6) as ocp,
        ):
            # Prime the exp-pool slots with zeros once: diagonal-tile exps
            # write only [delta:], and the 0/1 mask multiply reads the whole
            # tile — stale bytes must be finite (0 * garbage = 0 needs
            # garbage != NaN/Inf).
            prime = [ep.tile([128, QSL], BF16, tag="exp", name=f"prime{i}")
                     for i in range(8)]
            for pt_ in prime:
                nc.gpsimd.memset(pt_, 0.0)

            cq = []  # pending o_proj blocks: (qs, Dt)

            def emit_c_block():
                qs, Dt = cq.pop(0)
                qsl = slice(qs * QSL, (qs + 1) * QSL)
                pf = psC.tile([128, QSL], F32, tag="psC", name="psC")
                for g in range(HQ):
                    nc.tensor.matmul(
                        pf, wo_sb[:, g, Dt * 128:(Dt + 1) * 128],
                        o_attn[g][:, qsl],
                        start=(g == 0), stop=(g == HQ - 1))
                oc = ocp.tile([128, QSL], F32, tag="oc")
                nc.scalar.copy(oc, pf)
                nc.sync.dma_start(
                    out=outT[Dt * 128:(Dt + 1) * 128, qsl], in_=oc)

            for qs in range(NQS):
                qsl = slice(qs * QSL, (qs + 1) * QSL)
                nkt = (qs + 1) * (QSL // 128)
                # o_proj blocks of qs-1: spread most across this q-slice's
                # t-loop, hold back a few to cover the softmax-denominator
                # chain at the end of the slice.
                reserve = min(4, len(cq))
                bpt = max(0, (len(cq) - reserve)) // nkt if cq else 0
                po = [psO.tile([128, QSL], F32, tag="psO", name="psO")
                      for _ in range(HQ)]
                acc = [ap_.tile([128, QSL], F32, tag="acc", name="acc")
                       for _ in range(HQ)]
                accr = [arp.tile([128, QSL], F32R, tag="accr", name="accr")
                        for _ in range(HQ)]
                for t in range(nkt):
                    nc1 = (bpt + 1) // 2
                    exs = []
                    for g in range(HQ):
                        exs.append(emit_sexp(psS, "psS", qs, t, g))
                        if g == 1:  # PE filler while exps run
                            for _ in range(nc1):
                                if cq:
                                    emit_c_block()
                    for g in range(HQ):
                        nc.tensor.matmul(po[g], v_all[:, t, :], exs[g],
                                         start=(t == 0),
                                         stop=(t == nkt - 1))
                        if t == 0:
                            nc.vector.tensor_copy(acc[g], exs[g])
                        elif t == nkt - 1:
                            # final add rounds to f32r for the ones-matmul
                            nc.vector.tensor_add(accr[g], acc[g], exs[g])
                        elif g == 0:
                            nc.gpsimd.tensor_add(acc[g], acc[g], exs[g])
                        else:
                            nc.vector.tensor_add(acc[g], acc[g], exs[g])
                    for _ in range(bpt - nc1):
                        if cq:
                            emit_c_block()
                # softmax denominators + normalize into o_attn (bf16);
                # held-back o_proj blocks keep the PE fed meanwhile
                for g in range(HQ):
                    if cq:
                        emit_c_block()
                    pd = psS.tile([128, QSL], F32, tag="psS", name="pd")
                    nc.tensor.matmul(pd, ones_sb, accr[g],
                                     start=True, stop=True)
                    dn = dp.tile([128, QSL], F32, tag="dn")
                    nc.vector.reciprocal_approx_fast(dn, pd)
                    nc.vector.tensor_mul(o_attn[g][:, qsl], po[g], dn)
                cq.extend((qs, Dt) for Dt in range(NDT))
            while cq:
                emit_c_block()

    nc.compile()
    return nc


def make_tables(cfg: Cfg, position_ids: np.ndarray):
    """cosT/sinT [128, S]: row d holds cos/sin(pos * invfreq[d % 64])."""
    half = cfg.DH // 2
    inv = 1.0 / (cfg.theta ** (np.arange(half, dtype=np.float64) * 2.0 / cfg.DH))
    pos = np.asarray(position_ids).reshape(-1).astype(np.float64)  # [S]
    ang = inv[:, None] * pos[None, :]                              # [64, S]
    cosT = np.concatenate([np.cos(ang), np.cos(ang)], 0)
    sinT = np.concatenate([np.sin(ang), np.sin(ang)], 0)
    return cosT, sinT


def make_cst(cfg: Cfg):
    """[128, 256] cols 0-127: rotate-half stationary matrix (out = M^T @ x,
    out[:64] = -x[64:], out[64:] = x[:64]); cols 128-255: identity."""
    half = cfg.DH // 2
    m = np.zeros((128, 256), np.float64)
    for i in range(half):
        m[i + half, i] = -1.0
        m[i, i + half] = 1.0
    m[:, 128:256] = np.eye(128)
    return m


def make_binm(cfg: Cfg):
    """0/1 staircase [128, 2*QSL]: col c, row p -> 1 if (c - QSL) >= p else 0.
    Diagonal k-tile with offset delta uses cols [QSL-delta, 2*QSL-delta)."""
    c = np.arange(2 * cfg.QSL)[None, :] - cfg.QSL
    p = np.arange(128)[:, None]
    return (c >= p).astype(np.float64)


_cache = threading.Lock()
_nc_full = None


def _get_nc():
    global _nc_full
    with _cache:
        if _nc_full is None:
            _nc_full = build_nc(FULL)
    return _nc_full


def core_inputs(cfg: Cfg, c: int, position_ids, hidden_states, Wq, Wk, Wv, Wo):
    """Build the per-core input map (numpy, bf16 operands) for core c."""
    from ml_dtypes import bfloat16

    def bf(x):
        return np.ascontiguousarray(np.asarray(x).astype(bfloat16))

    S, D, HQ, DH = cfg.S, cfg.D, cfg.HQ, cfg.DH
    hT = np.asarray(hidden_states, dtype=np.float32).reshape(S, D).T
    cosT, sinT = make_tables(cfg, position_ids)
    qc = slice(c * HQ * DH, (c + 1) * HQ * DH)
    kc = slice(c * DH, (c + 1) * DH)
    return {
        "hT": bf(hT),
        "wq": bf(np.asarray(Wq, np.float32)[:, qc]),
        "wk": bf(np.asarray(Wk, np.float32)[:, kc]),
        "wv": bf(np.asarray(Wv, np.float32)[:, kc]),
        "wo": bf(np.asarray(Wo, np.float32)[qc, :]),
        "cosT": bf(cosT),
        "sinT": bf(sinT),
        "binm": bf(make_binm(cfg)),
        "cstb": bf(make_cst(cfg)),
        "onesf": np.ones((128, 128), np.float32),
    }


def kernel(position_ids, hidden_states, Wq, Wk, Wv, Wo, _trace=False):
    from concourse.bass_utils import run_bass_kernel_spmd

    cfg = FULL
    nc = _get_nc()
    args = (position_ids, hidden_states, Wq, Wk, Wv, Wo)
    in_maps = [core_inputs(cfg, c, *args) for c in range(cfg.cores)]
    res = run_bass_kernel_spmd(nc, in_maps, core_ids=list(range(cfg.cores)),
                               trace=_trace)
    out = np.zeros((cfg.S, cfg.D), np.float64)
    for c in range(cfg.cores):
        out += res.results[c]["outT"].T.astype(np.float64)
    ret = out.astype(np.float32).reshape(1, cfg.S, cfg.D)
    if _trace:
        return ret, res
    return ret
